# revision 8
# baseline (speedup 1.0000x reference)
"""MoE top-1 routing kernel for Trainium2 (8 NeuronCores, expert-parallel).

Model (E=8, D=512, F=2048, N=4096):
    logits = x @ Wg + bg; e = argmax(logits)
    y[i] = relu(x[i] @ W1[e] + b1[e]) @ W2[e] + b2[e]

Strategy (v2, fp16):
- Host computes the gate (f64 matmul + argmax) and routes tokens; core e gets
  only expert e's tokens (padded to the max expert count C) + expert e's
  weights, and runs a dense 2-layer MLP with fp16 operands (weights, x, h) and
  fp32 PSUM accumulation (~3e-4 rel err vs f64 oracle, threshold 2e-2).
- fp16 halves HBM traffic vs fp32r (4.7MB/core total) and avoids the fp32r
  LOW_HIGH double-pass on the PE.
- All tensors are host-packed into SBUF-native [128, *] layouts; DMA pieces
  are issued on the sync engine in consumption order, with the first pieces
  (bias, w1 ko0 slice, x chunk-A ko0 slice) kept small so real matmuls start
  as soon as the engines boot (~7.5us).
- A short warm-up burst (6 thin fp16 matmuls on a memset tile) keeps the PE
  busy from engine-boot until real data lands, so the HAM clock-gate ramps to
  2.4GHz early; the burst is thin enough not to delay real work.
- Stage-1 waves are emitted ko0-first (4 matmuls needing only the first DMA
  pieces), then f-major ko1-3 + ReLU activation per f-tile; 4 PSUM banks.
- Stage-2 is d-major with 2 double-buffered PSUM tags; each d's bias-add
  (vector) and output DMA (scalar, fp16) streams while the PE continues.
- Tile/tag/queue counts are kept minimal: the TileContext teardown (semaphore
  resets) lands inside the measured window, so fewer semaphores = faster.
"""

import sys

sys.path.insert(0, "/opt/trn_rl_repo")

import numpy as np

E, D, F, N_CORES = 8, 512, 2048, 8
KD, KF = D // 128, F // 128  # 4, 16
G1, G2 = KF // 4, KF // 4    # w1 f-group count, w2 fo-group count (4 each)

_cache: dict = {}


def _build(C: int, chunks: list[tuple[int, int]]):
    import concourse.tile as tile
    import concourse.mybir as mybir
    from concourse import bacc

    f16, f32 = mybir.dt.float16, mybir.dt.float32
    Relu = mybir.ActivationFunctionType.Relu

    nc = bacc.Bacc("TRN2", target_bir_lowering=False, debug=False)
    # NOTE: m.queues num_queues=16 is the SDMA engine fan-out per dma_start
    # (16 engines); shrinking it caps DMA bandwidth (~27GB/s per engine).
    # packed layouts, all [128, *], fp16 except biases/psum:
    #   xTi[p, chunk_off + ko*cw + c] = x_e[c0+c, 128*ko+p]
    #   w1i[p, g*2048 + ko*512 + fi]  = W1_e[128*ko+p, 512*g+fi]
    #   w2i[p, h*2048 + j*512 + d]    = W2_e[128*(4h+j)+p, d]
    #   bi[p, f] = b1_e[128f+p] (f<16);  bi[p, 16+d] = b2_e[128d+p]
    #   yTi[p, d*C + c] = y_e[c, 128d+p]
    xTi = nc.dram_tensor("xTi", [128, KD * C], f16, kind="ExternalInput").ap()
    w1i = nc.dram_tensor("w1i", [128, KD * F], f16, kind="ExternalInput").ap()
    w2i = nc.dram_tensor("w2i", [128, KF * D], f16, kind="ExternalInput").ap()
    bi = nc.dram_tensor("bi", [128, KF + KD], f32, kind="ExternalInput").ap()
    yTi = nc.dram_tensor("yTi", [128, KD * C], f16, kind="ExternalOutput").ap()

    (a0, a1) = chunks[0]
    cwA = a1 - a0

    with tile.TileContext(nc) as tc:
        with tc.tile_pool(name="sb", bufs=1) as sb, \
             tc.tile_pool(name="pp", bufs=1, space="PSUM") as pp:

            # --- tiles ---
            bis = sb.tile([128, KF + KD], f32, name="bis")
            w1t = sb.tile([128, KD * F], f16, name="w1t")
            w2t = sb.tile([128, KF * D], f16, name="w2t")
            xt = sb.tile([128, KD * C], f16, name="xt")
            ht = sb.tile([128, KF * C], f16, name="ht")
            ys = sb.tile([128, KD * C], f16, name="ys")
            wrm = sb.tile([128, 128], f16, name="wrm")

            p1 = [pp.tile([128, cwA], f32, name=f"p1_{fl}", tag=f"p1_{fl}",
                          bufs=1) for fl in range(4)]

            # --- PE warm-up: thin fp16 matmuls from engine-boot (~6.8us)
            # until the first w1/x pieces' completion sems fire (~11.5us).
            # Any PE idle gap >~1us here resets the HAM busy window and
            # postpones the 2.4GHz ramp, costing ~3us of half-clock stage 1.
            nc.vector.memset(wrm[:], 0.0)
            for _ in range(22):
                nc.tensor.matmul(p1[0][:, 0:128], wrm[:], wrm[:],
                                 start=True, stop=True)

            # --- DMA: one HWDGE queue (sync), consumption order, >=312KB
            # pieces. The early issue rate (~0.65us/piece) is the delivery
            # limiter, so big-first maximizes early bytes; parallel queues
            # interleave pathologically (w2 bytes steal from w1g0) — don't.
            # The tiny bias piece rides the otherwise-idle scalar queue.
            nc.scalar.dma_start(bis[:], bi[:])
            nc.sync.dma_start(w1t[:, 0:2048], w1i[:, 0:2048])            # g0
            nc.sync.dma_start(xt[:, 0:KD * cwA], xTi[:, 0:KD * cwA])     # A
            nc.sync.dma_start(xt[:, KD * cwA:KD * C], xTi[:, KD * cwA:KD * C])
            for g in range(1, G1):
                nc.sync.dma_start(w1t[:, g * 2048:(g + 1) * 2048],
                                  w1i[:, g * 2048:(g + 1) * 2048])
            nc.sync.dma_start(w2t[:, 0:4096], w2i[:, 0:4096])
            nc.sync.dma_start(w2t[:, 4096:8192], w2i[:, 4096:8192])

            # --- stage 1: h = relu(x @ W1 + b1) ---
            for g in range(G1):
                for ci, (c0, c1) in enumerate(chunks):
                    cw = c1 - c0
                    xoff = KD * c0
                    pw = [pp.tile([128, cwA], f32, name=f"p1_{g}_{ci}_{fl}",
                                  tag=f"p1_{fl}", bufs=1) for fl in range(4)]
                    for fl in range(4):
                        f = 4 * g + fl
                        for ko in range(KD):
                            nc.tensor.matmul(
                                pw[fl][:, 0:cw],
                                w1t[:, g * 2048 + ko * 512 + fl * 128:
                                    g * 2048 + ko * 512 + fl * 128 + 128],
                                xt[:, xoff + ko * cw:xoff + (ko + 1) * cw],
                                start=(ko == 0), stop=(ko == KD - 1))
                        nc.scalar.activation(ht[:, f * C + c0:f * C + c1],
                                             pw[fl][:, 0:cw], Relu,
                                             bias=bis[:, f:f + 1])

            # --- stage 2: y = h @ W2 + b2, d-major; output streams out ---
            for d in range(KD):
                for ci, (c0, c1) in enumerate(chunks):
                    cw = c1 - c0
                    p2 = pp.tile([128, cwA], f32, name=f"p2_{d}_{ci}",
                                 tag=f"p2_{d % 2}", bufs=2)
                    for fo in range(KF):
                        h2, j = divmod(fo, 4)
                        nc.tensor.matmul(
                            p2[:, 0:cw],
                            w2t[:, h2 * 2048 + j * 512 + d * 128:
                                h2 * 2048 + j * 512 + d * 128 + 128],
                            ht[:, fo * C + c0:fo * C + c1],
                            start=(fo == 0), stop=(fo == KF - 1))
                    nc.vector.tensor_scalar_add(ys[:, d * C + c0:d * C + c1],
                                                p2[:, 0:cw],
                                                bis[:, KF + d:KF + d + 1])
                if d % 2 == 1:
                    nc.scalar.dma_start(yTi[:, (d - 1) * C:(d + 1) * C],
                                        ys[:, (d - 1) * C:(d + 1) * C])
    nc.compile()
    return nc


def _plan_chunks(C: int) -> list[tuple[int, int]]:
    n = max(1, -(-C // 512))
    base, rem = divmod(C, n)
    out, pos = [], 0
    for i in range(n):
        w = base + (1 if i < rem else 0)
        out.append((pos, pos + w))
        pos += w
    return out


def _get_nc(C: int):
    if C not in _cache:
        _cache[C] = _build(C, _plan_chunks(C))
    return _cache[C]


def _pack_inputs(x, W1, b1, W2, b2, idx, order, starts, C):
    chunks = _plan_chunks(C)
    in_maps, toks_per_core = [], []
    for e in range(E):
        toks = order[starts[e]:starts[e + 1]]
        toks_per_core.append(toks)
        xe = np.zeros((C, D), np.float16)
        xe[:len(toks)] = x[toks]
        xeT = xe.T  # [D, C]
        xTi = np.concatenate(
            [xeT[:, c0:c1].reshape(KD, 128, c1 - c0).transpose(1, 0, 2)
             .reshape(128, KD * (c1 - c0)) for c0, c1 in chunks], axis=1)
        w1p = np.concatenate(
            [W1[e][:, 512 * g:512 * (g + 1)].astype(np.float16)
             .reshape(KD, 128, 512)
             .transpose(1, 0, 2).reshape(128, KD * 512) for g in range(G1)], axis=1)
        w2p = np.concatenate(
            [W2[e][512 * h:512 * (h + 1), :].astype(np.float16)
             .reshape(4, 128, 512)
             .transpose(1, 0, 2).reshape(128, 4 * 512) for h in range(G2)], axis=1)
        bi = np.concatenate([b1[e].reshape(KF, 128).T,
                             b2[e].reshape(KD, 128).T], axis=1).astype(np.float32)
        in_maps.append({
            "xTi": np.ascontiguousarray(xTi),
            "w1i": np.ascontiguousarray(w1p),
            "w2i": np.ascontiguousarray(w2p),
            "bi": np.ascontiguousarray(bi),
        })
    return in_maps, toks_per_core, chunks


def kernel(x, Wg, bg, W1, b1, W2, b2):
    from concourse.bass_utils import run_bass_kernel_spmd

    x = np.asarray(x, dtype=np.float32)
    n_tok = x.shape[0]

    # host gate in f64: the mathematically-true argmax
    logits = x.astype(np.float64) @ np.asarray(Wg, np.float64) + np.asarray(bg, np.float64)
    idx = logits.argmax(1)

    counts = np.bincount(idx, minlength=E)
    order = np.argsort(idx, kind="stable")
    starts = np.zeros(E + 1, np.int64)
    starts[1:] = np.cumsum(counts)

    C = max(int(counts.max()), 256)
    C = (C + 15) // 16 * 16

    W1 = np.asarray(W1, np.float32)
    W2 = np.asarray(W2, np.float32)
    b1 = np.asarray(b1, np.float32)
    b2 = np.asarray(b2, np.float32)

    in_maps, toks_per_core, chunks = _pack_inputs(x, W1, b1, W2, b2,
                                                  idx, order, starts, C)
    nc = _get_nc(C)
    res = run_bass_kernel_spmd(nc, in_maps, core_ids=list(range(N_CORES)))

    out = np.zeros((n_tok, D), np.float32)
    for e in range(E):
        toks = toks_per_core[e]
        ye = res.results[e]["yTi"].reshape(128, KD, C).transpose(2, 1, 0) \
            .reshape(C, D).astype(np.float32)
        out[toks] = ye[:len(toks)]
    return out


# revision 11
# speedup vs baseline: 1.0228x; 1.0228x over previous
"""MoE top-1 routing kernel for Trainium2 (8 NeuronCores, expert-parallel).

Model (E=8, D=512, F=2048, N=4096):
    logits = x @ Wg + bg; e = argmax(logits)
    y[i] = relu(x[i] @ W1[e] + b1[e]) @ W2[e] + b2[e]

Strategy (v2, fp16):
- Host computes the gate (f64 matmul + argmax) and routes tokens; core e gets
  only expert e's tokens (padded to the max expert count C) + expert e's
  weights, and runs a dense 2-layer MLP with fp16 operands (weights, x, h) and
  fp32 PSUM accumulation (~3e-4 rel err vs f64 oracle, threshold 2e-2).
- fp16 halves HBM traffic vs fp32r (4.7MB/core total) and avoids the fp32r
  LOW_HIGH double-pass on the PE.
- All tensors are host-packed into SBUF-native [128, *] layouts; DMA pieces
  are issued on the sync engine in consumption order, with the first pieces
  (bias, w1 ko0 slice, x chunk-A ko0 slice) kept small so real matmuls start
  as soon as the engines boot (~7.5us).
- A short warm-up burst (6 thin fp16 matmuls on a memset tile) keeps the PE
  busy from engine-boot until real data lands, so the HAM clock-gate ramps to
  2.4GHz early; the burst is thin enough not to delay real work.
- Stage-1 waves are emitted ko0-first (4 matmuls needing only the first DMA
  pieces), then f-major ko1-3 + ReLU activation per f-tile; 4 PSUM banks.
- Stage-2 is d-major with 2 double-buffered PSUM tags; each d's bias-add
  (vector) and output DMA (scalar, fp16) streams while the PE continues.
- Tile/tag/queue counts are kept minimal: the TileContext teardown (semaphore
  resets) lands inside the measured window, so fewer semaphores = faster.
"""

import sys

sys.path.insert(0, "/opt/trn_rl_repo")

import numpy as np

E, D, F, N_CORES = 8, 512, 2048, 8
KD, KF = D // 128, F // 128  # 4, 16
G1, G2 = KF // 4, KF // 4    # w1 f-group count, w2 fo-group count (4 each)

_cache: dict = {}


def _build(C: int, chunks: list[tuple[int, int]]):
    import concourse.tile as tile
    import concourse.mybir as mybir
    from concourse import bacc

    f16, f32 = mybir.dt.float16, mybir.dt.float32
    Relu = mybir.ActivationFunctionType.Relu

    nc = bacc.Bacc("TRN2", target_bir_lowering=False, debug=False)
    # NOTE: m.queues num_queues=16 is the SDMA engine fan-out per dma_start
    # (16 engines); shrinking it caps DMA bandwidth (~27GB/s per engine).
    # packed layouts, all [128, *], fp16 except biases/psum:
    #   xTi[p, chunk_off + ko*cw + c] = x_e[c0+c, 128*ko+p]
    #   w1i[p, g*2048 + ko*512 + fi]  = W1_e[128*ko+p, 512*g+fi]
    #   w2i[p, h*2048 + j*512 + d]    = W2_e[128*(4h+j)+p, d]
    #   bi[p, f] = b1_e[128f+p] (f<16);  bi[p, 16+d] = b2_e[128d+p]
    #   yTi[p, d*C + c] = y_e[c, 128d+p]
    xTi = nc.dram_tensor("xTi", [128, KD * C], f16, kind="ExternalInput").ap()
    w1i = nc.dram_tensor("w1i", [128, KD * F], f16, kind="ExternalInput").ap()
    w2i = nc.dram_tensor("w2i", [128, KF * D], f16, kind="ExternalInput").ap()
    bi = nc.dram_tensor("bi", [128, KF + KD], f32, kind="ExternalInput").ap()
    yTi = nc.dram_tensor("yTi", [128, KD * C], f16, kind="ExternalOutput").ap()

    (a0, a1) = chunks[0]
    cwA = a1 - a0

    with tile.TileContext(nc) as tc:
        with tc.tile_pool(name="sb", bufs=1) as sb, \
             tc.tile_pool(name="pp", bufs=1, space="PSUM") as pp:

            # --- tiles ---
            bis = sb.tile([128, KF + KD], f32, name="bis")
            w1t = sb.tile([128, KD * F], f16, name="w1t")
            w2t = sb.tile([128, KF * D], f16, name="w2t")
            xt = sb.tile([128, KD * C], f16, name="xt")
            ht = sb.tile([128, KF * C], f16, name="ht")
            ys = sb.tile([128, KD * C], f16, name="ys")
            wrm = sb.tile([128, 128], f16, name="wrm")

            p1 = [pp.tile([128, cwA], f32, name=f"p1_{fl}", tag=f"p1_{fl}",
                          bufs=1) for fl in range(4)]

            # --- PE warm-up: thin fp16 matmuls from engine-boot (~6.8us)
            # until the first w1/x pieces' completion sems fire (~11.5us).
            # Any PE idle gap >~1us here resets the HAM busy window and
            # postpones the 2.4GHz ramp, costing ~3us of half-clock stage 1.
            nc.vector.memset(wrm[:], 0.0)
            for _ in range(30):
                nc.tensor.matmul(p1[0][:, 0:128], wrm[:], wrm[:],
                                 start=True, stop=True)

            # --- DMA: one HWDGE queue (sync), consumption order, >=312KB
            # pieces. The early issue rate (~0.65us/piece) is the delivery
            # limiter, so big-first maximizes early bytes; parallel queues
            # interleave pathologically (w2 bytes steal from w1g0) — don't.
            # The tiny bias piece rides the otherwise-idle scalar queue.
            nc.scalar.dma_start(bis[:], bi[:])
            nc.sync.dma_start(w1t[:, 0:1024], w1i[:, 0:1024])        # g0 k01
            nc.sync.dma_start(xt[:, 0:KD * cwA], xTi[:, 0:KD * cwA])     # A
            nc.sync.dma_start(w1t[:, 1024:2048], w1i[:, 1024:2048])  # g0 k23
            nc.sync.dma_start(xt[:, KD * cwA:KD * C], xTi[:, KD * cwA:KD * C])
            for g in range(1, G1):
                nc.sync.dma_start(w1t[:, g * 2048:(g + 1) * 2048],
                                  w1i[:, g * 2048:(g + 1) * 2048])
            nc.sync.dma_start(w2t[:, 0:4096], w2i[:, 0:4096])
            nc.sync.dma_start(w2t[:, 4096:8192], w2i[:, 4096:8192])

            # --- stage 1: h = relu(x @ W1 + b1) ---
            for g in range(G1):
                for ci, (c0, c1) in enumerate(chunks):
                    cw = c1 - c0
                    xoff = KD * c0
                    pw = [pp.tile([128, cwA], f32, name=f"p1_{g}_{ci}_{fl}",
                                  tag=f"p1_{fl}", bufs=1) for fl in range(4)]
                    for fl in range(4):
                        f = 4 * g + fl
                        for ko in range(KD):
                            nc.tensor.matmul(
                                pw[fl][:, 0:cw],
                                w1t[:, g * 2048 + ko * 512 + fl * 128:
                                    g * 2048 + ko * 512 + fl * 128 + 128],
                                xt[:, xoff + ko * cw:xoff + (ko + 1) * cw],
                                start=(ko == 0), stop=(ko == KD - 1))
                        nc.scalar.activation(ht[:, f * C + c0:f * C + c1],
                                             pw[fl][:, 0:cw], Relu,
                                             bias=bis[:, f:f + 1])

            # --- stage 2: y = h @ W2 + b2, d-major; output streams out ---
            for d in range(KD):
                for ci, (c0, c1) in enumerate(chunks):
                    cw = c1 - c0
                    p2 = pp.tile([128, cwA], f32, name=f"p2_{d}_{ci}",
                                 tag=f"p2_{d % 2}", bufs=2)
                    for fo in range(KF):
                        h2, j = divmod(fo, 4)
                        nc.tensor.matmul(
                            p2[:, 0:cw],
                            w2t[:, h2 * 2048 + j * 512 + d * 128:
                                h2 * 2048 + j * 512 + d * 128 + 128],
                            ht[:, fo * C + c0:fo * C + c1],
                            start=(fo == 0), stop=(fo == KF - 1))
                    nc.vector.tensor_scalar_add(ys[:, d * C + c0:d * C + c1],
                                                p2[:, 0:cw],
                                                bis[:, KF + d:KF + d + 1])
                if d == 1:
                    nc.scalar.dma_start(yTi[:, 0:2 * C], ys[:, 0:2 * C])
                elif d > 1:
                    nc.scalar.dma_start(yTi[:, d * C:(d + 1) * C],
                                        ys[:, d * C:(d + 1) * C])
    nc.compile()
    return nc


def _plan_chunks(C: int) -> list[tuple[int, int]]:
    n = max(1, -(-C // 512))
    base, rem = divmod(C, n)
    out, pos = [], 0
    for i in range(n):
        w = base + (1 if i < rem else 0)
        out.append((pos, pos + w))
        pos += w
    return out


def _get_nc(C: int):
    if C not in _cache:
        _cache[C] = _build(C, _plan_chunks(C))
    return _cache[C]


def _pack_inputs(x, W1, b1, W2, b2, idx, order, starts, C):
    chunks = _plan_chunks(C)
    in_maps, toks_per_core = [], []
    for e in range(E):
        toks = order[starts[e]:starts[e + 1]]
        toks_per_core.append(toks)
        xe = np.zeros((C, D), np.float16)
        xe[:len(toks)] = x[toks]
        xeT = xe.T  # [D, C]
        xTi = np.concatenate(
            [xeT[:, c0:c1].reshape(KD, 128, c1 - c0).transpose(1, 0, 2)
             .reshape(128, KD * (c1 - c0)) for c0, c1 in chunks], axis=1)
        w1p = np.concatenate(
            [W1[e][:, 512 * g:512 * (g + 1)].astype(np.float16)
             .reshape(KD, 128, 512)
             .transpose(1, 0, 2).reshape(128, KD * 512) for g in range(G1)], axis=1)
        w2p = np.concatenate(
            [W2[e][512 * h:512 * (h + 1), :].astype(np.float16)
             .reshape(4, 128, 512)
             .transpose(1, 0, 2).reshape(128, 4 * 512) for h in range(G2)], axis=1)
        bi = np.concatenate([b1[e].reshape(KF, 128).T,
                             b2[e].reshape(KD, 128).T], axis=1).astype(np.float32)
        in_maps.append({
            "xTi": np.ascontiguousarray(xTi),
            "w1i": np.ascontiguousarray(w1p),
            "w2i": np.ascontiguousarray(w2p),
            "bi": np.ascontiguousarray(bi),
        })
    return in_maps, toks_per_core, chunks


def kernel(x, Wg, bg, W1, b1, W2, b2):
    from concourse.bass_utils import run_bass_kernel_spmd

    x = np.asarray(x, dtype=np.float32)
    n_tok = x.shape[0]

    # host gate in f64: the mathematically-true argmax
    logits = x.astype(np.float64) @ np.asarray(Wg, np.float64) + np.asarray(bg, np.float64)
    idx = logits.argmax(1)

    counts = np.bincount(idx, minlength=E)
    order = np.argsort(idx, kind="stable")
    starts = np.zeros(E + 1, np.int64)
    starts[1:] = np.cumsum(counts)

    C = max(int(counts.max()), 256)
    C = (C + 15) // 16 * 16

    W1 = np.asarray(W1, np.float32)
    W2 = np.asarray(W2, np.float32)
    b1 = np.asarray(b1, np.float32)
    b2 = np.asarray(b2, np.float32)

    in_maps, toks_per_core, chunks = _pack_inputs(x, W1, b1, W2, b2,
                                                  idx, order, starts, C)
    nc = _get_nc(C)
    res = run_bass_kernel_spmd(nc, in_maps, core_ids=list(range(N_CORES)))

    out = np.zeros((n_tok, D), np.float32)
    for e in range(E):
        toks = toks_per_core[e]
        ye = res.results[e]["yTi"].reshape(128, KD, C).transpose(2, 1, 0) \
            .reshape(C, D).astype(np.float32)
        out[toks] = ye[:len(toks)]
    return out
